# revision 35
# baseline (speedup 1.0000x reference)
"""CTC greedy decoder on 8 TRN2 NeuronCores (Bass/Tile).

Problem: x (2000, 32, 1024) f32 logits, lengths (32,) i32.
Output: tokens (32, 2000) i32 (left-compacted, -1 padded), out_lengths (32,) i32.

Sharding: data-parallel over batch. Each of the 8 cores gets 4 batch rows
(x[:, 4i:4i+4, :], lengths[4i:4i+4]) and decodes them independently.

Per-core algorithm:
  Phase 1 (heavy): argmax over the 1024-wide alphabet for all 2000*4
    positions. The max pass runs on GPSIMD (tensor_reduce), the index
    pass on DVE (max_index with a stride-0 broadcast of the max), and the
    512KB x-tile loads alternate between the two HWDGE rings (SP/ACT).
    Timesteps are tiled as t = p*16 + q (p: 125 partitions, q: 16 tile
    columns per batch row) so "previous timestep" is a free-dim shift.
  Phase 2 (light, per batch row): CTC keep-mask (valid & nonblank &
    non-repeat), mask dropped positions to -1, transpose to the 16-minor
    wrapped layout. The gpsimd.sparse_gather compactions (+ tail -1
    rebuild, since HW leaves garbage past num_found) run after all
    reduces so the GPSIMD library is switched exactly once.
"""

import numpy as np
import sys

sys.path.insert(0, "/opt/trn_rl_repo")

import concourse.bass as bass
import concourse.mybir as mybir
from concourse import bacc
from concourse import dve_ops
from concourse.dve_spec import Spec, Src0, Idx, MaxNeg, select, scan, AluOp
from concourse.dve_spec import lower as dve_lower
from concourse.dve_uop import DveOpSpec
from concourse.tile import TileContext
from concourse.bass_utils import run_bass_kernel_spmd


def _argmax_rev_ref(in0, in1, s0, s1, imm2):
    """CoreSim reference: in0 arrives through the caller's (reversed) AP."""
    y = in0.astype(np.float32).reshape(in0.shape[0], -1)
    r = np.maximum.accumulate(y, axis=1)
    body = np.where(y >= r, np.arange(y.shape[1], dtype=np.float32), -3.4028235e38)
    return body.astype(np.float32), body.max(axis=1, keepdims=True).astype(np.float32)


def _register_argmax_op():
    """One-pass argmax DVE op: stream position of the LAST running-max event.

    Called on a reversed view of the row, the last event is the FIRST
    occurrence of the row maximum in true order, so
    true_argmax = (N-1) - accum_out. Exact for ties, one 1x pass.
    """
    name = "ARGMAX_REV_ANT"
    if any(op.name == name for op in dve_ops.OPS):
        return next(op for op in dve_ops.OPS if op.name == name)
    spec = Spec(
        body=select(Src0 >= scan(AluOp.MAX, Src0), Idx, MaxNeg),
        accum=AluOp.MAX,
        reference=_argmax_rev_ref,
    )
    shas = {}
    for ver in ("v3", "v4"):
        try:
            sp = DveOpSpec(name=name, opcode=0, uops=dve_lower(spec, ver=ver))
            shas[ver] = sp.sha(ver)
        except Exception:
            pass
    op = dve_ops.DveOp(name, spec, subdim=False, uops_sha=shas)
    row = dve_ops._CUSTOM_DVE_ROW_BASE + len(dve_ops.OPS)
    assert row < 0x20
    dve_ops.OPS.append(op)
    dve_ops._SUB_OPCODE_FOR_NAME[name] = row
    dve_ops.CUSTOM_DVE_SPECS[name] = spec
    return op


ARGMAX_REV = _register_argmax_op()

# Problem constants (hardcoded; kernel.py must be self-contained).
T = 2000          # sequence length
B = 32            # global batch
A = 1024          # alphabet size
NCORES = 8
BL = B // NCORES  # batch rows per core = 4
Q = 16            # tile columns per batch row (t = p*Q + q)
P = T // Q        # used partitions per tile = 125

F32 = mybir.dt.float32
I32 = mybir.dt.int32
U32 = mybir.dt.uint32

_CACHE = {}


def _build_nc():
    """Build the single-core Bass graph (same graph runs SPMD on all cores)."""
    nc = bacc.Bacc(num_swdge_queues=2)

    # x arrives host-rearranged as [g, b, p, s, a] with t = p*16 + g*4 + s,
    # so each 8-column tile load is one fully contiguous 4MB block.
    x_d = nc.declare_dram_parameter("x", [2, BL, P, 8, A], F32, isOutput=False)
    len_d = nc.declare_dram_parameter("lengths", [1, BL], I32, isOutput=False)
    ident_d = nc.declare_dram_parameter("c_ident", [128, 128], F32, isOutput=False)
    shift_d = nc.declare_dram_parameter("c_shift", [128, 128], F32, isOutput=False)
    iota_d = nc.declare_dram_parameter("c_iota", [128, Q], F32, isOutput=False)
    iotaj_d = nc.declare_dram_parameter("c_iotaj", [Q, 128], F32, isOutput=False)
    tok_d = nc.declare_dram_parameter("tokens", [BL, T], I32, isOutput=True)
    lout_d = nc.declare_dram_parameter("lens_out", [BL, 1], I32, isOutput=True)

    xv4 = x_d
    # tokens[b, j] with j = p*16 + q: tokv[b, p, q]
    tokv = tok_d.rearrange("b (p s) -> b p s", s=Q)

    with TileContext(nc) as tc:
        with (
            tc.tile_pool(name="consts", bufs=1) as cpool,
            tc.tile_pool(name="xp", bufs=4) as xpool,
            tc.tile_pool(name="fp", bufs=4) as fpool,
            tc.tile_pool(name="p2", bufs=4) as p2pool,
            tc.tile_pool(name="sg", bufs=4) as sgpool,
            tc.tile_pool(name="ps", bufs=2, space="PSUM") as pspool,
        ):
            # ---- constants ----
            ident = cpool.tile([128, 128], F32)
            nc.sync.dma_start(ident[:], ident_d[:])
            shiftm = cpool.tile([128, 128], F32)
            nc.sync.dma_start(shiftm[:], shift_d[:])
            iota_t = cpool.tile([128, Q], F32)
            nc.sync.dma_start(iota_t[:], iota_d[:])
            neg1 = cpool.tile([128, Q], F32)
            nc.vector.memset(neg1[:], -1.0)
            ones_k = cpool.tile([1, 128], F32)
            nc.vector.memset(ones_k[:], 1.0)
            iota_j = cpool.tile([Q, 128], F32)
            nc.sync.dma_start(iota_j[:], iotaj_d[:])
            neg1w = cpool.tile([Q, 128], F32)
            nc.vector.memset(neg1w[:], -1.0)

            # lengths -> f32, broadcast to all 128 partitions via K=1 matmul
            len_raw = cpool.tile([1, BL], I32)
            nc.sync.dma_start(len_raw[:], len_d[:])
            len_f = cpool.tile([1, BL], F32)
            nc.vector.tensor_copy(len_f[:], len_raw[:])
            len_ps = pspool.tile([128, BL], F32, tag="ps_a")
            nc.tensor.matmul(len_ps[:], ones_k[:], len_f[:])
            len_sb = cpool.tile([128, BL], F32)
            nc.vector.tensor_copy(len_sb[:], len_ps[:])

            sg_ins = []
            for b in range(BL):
                # ---- phase 1: argmax ----
                F = fpool.tile([128, Q], F32)
                nc.vector.memset(F[96:128, :], 0.0)  # covers the 3 unused rows
                for g in range(2):
                    # one SWDGE load brings 8 q-columns (32KB/partition runs),
                    # fanning out across all 16 SDMA engines; alternate between
                    # the two SWDGE descriptor rings
                    xt8 = xpool.tile([128, 8, A], F32)
                    ld = nc.gpsimd.dma_start(xt8[0:P, :, :], xv4[g, b])
                    if (b * 2 + g) % 2 == 1:
                        ld.ins.queue = "qPoolDynamic1"
                    for s in range(8):
                        q = g * 8 + s
                        # one-pass argmax: reversed stream, accum = last
                        # running-max event = first max occurrence (reversed)
                        xr = xt8[0:P, s, ::-1]
                        nc.vector._custom_dve(
                            ARGMAX_REV, out=xr, in0=xr,
                            accum_out=F[0:P, q : q + 1],
                        )
                # F holds (A-1) - argmax; flip to true index
                nc.vector.tensor_scalar(
                    F[:], F[:], -1.0, float(A - 1),
                    mybir.AluOpType.mult, mybir.AluOpType.add,
                )

                # ---- phase 2a: CTC keep-mask (DVE/PE only) ----
                # prev of column 0 = F[p-1, 15]: shift matmul
                prev0 = pspool.tile([128, 1], F32, tag="ps_b")
                nc.tensor.matmul(prev0[:], shiftm[:], F[:, Q - 1 : Q])

                neq = p2pool.tile([128, Q], F32)
                nc.vector.tensor_tensor(
                    neq[:, 1:Q], F[:, 1:Q], F[:, 0 : Q - 1], mybir.AluOpType.not_equal
                )
                nc.vector.tensor_tensor(
                    neq[:, 0:1], F[:, 0:1], prev0[:, 0:1], mybir.AluOpType.not_equal
                )
                nc.vector.memset(neq[0:1, 0:1], 1.0)  # t=0: prev is None

                nz = p2pool.tile([128, Q], F32)
                nc.vector.tensor_scalar(
                    nz[:], F[:], 0.0, None, mybir.AluOpType.not_equal
                )
                valid = p2pool.tile([128, Q], F32)
                nc.vector.tensor_tensor(
                    valid[:], iota_t[:],
                    len_sb[:, b : b + 1].broadcast_to([128, Q]),
                    mybir.AluOpType.is_lt,
                )
                keep = p2pool.tile([128, Q], F32)
                nc.vector.tensor_tensor(keep[:], neq[:], nz[:], mybir.AluOpType.mult)
                nc.vector.tensor_tensor(
                    keep[:], keep[:], valid[:], mybir.AluOpType.mult
                )

                keepi = p2pool.tile([128, Q], mybir.dt.uint8)
                nc.vector.tensor_copy(keepi[:], keep[:])
                masked = p2pool.tile([128, Q], F32)
                nc.vector.select(masked[:], keepi[:], F[:], neg1[:])

                # wrap to [16, 128] (t = f*16 + q order)
                tr_ps = pspool.tile([Q, 128], F32, tag="ps_a")
                nc.tensor.transpose(tr_ps[:], masked[:], ident[:])
                sg_in = sgpool.tile([Q, 128], F32, tag=f"sgin{b}")
                nc.vector.tensor_copy(sg_in[:], tr_ps[:])
                sg_ins.append(sg_in)

            # ---- phase 2b: compaction at the end, so sparse_gather never
            # blocks SWDGE load generation on the gpsimd stream ----
            for b in range(BL):
                sg_in = sg_ins[b]
                sg_out = sgpool.tile([Q, 128], F32)
                nf = sgpool.tile([1, 1], U32)
                nc.gpsimd.sparse_gather(sg_out[:], sg_in[:], num_found=nf[:])

                # The HW ucode writes deterministic garbage past num_found
                # (sim pads -1): rebuild the tail as -1 via j < num_found.
                nf_f = sgpool.tile([1, 1], F32)
                nc.vector.tensor_copy(nf_f[:], nf[:])
                nfbc_ps = pspool.tile([Q, 1], F32, tag="ps_b")
                nc.tensor.matmul(nfbc_ps[:], ones_k[0:1, 0:Q], nf_f[:])
                padm = sgpool.tile([Q, 128], mybir.dt.uint8)
                nc.vector.tensor_tensor(
                    padm[:], iota_j[:],
                    nfbc_ps[:, 0:1].broadcast_to([Q, 128]),
                    mybir.AluOpType.is_lt,
                )
                padded = sgpool.tile([Q, 128], F32)
                nc.vector.tensor_copy(padded[:], neg1w[:])
                nc.vector.copy_predicated(padded[:], padm[:], sg_out[:])

                # unwrap back to [128, 16] (j = p*16 + q) and store
                tok_ps = pspool.tile([128, Q], F32, tag="ps_c")
                nc.tensor.transpose(tok_ps[:], padded[:], ident[0:Q, 0:Q])
                toki = p2pool.tile([128, Q], I32)
                nc.vector.tensor_copy(toki[:], tok_ps[:])
                nc.sync.dma_start(tokv[b], toki[0:P, :])

                nf_i = sgpool.tile([1, 1], I32)
                nc.vector.tensor_copy(nf_i[:], nf[:])
                nc.sync.dma_start(lout_d[b : b + 1, :], nf_i[:])

    # Bacc.compile runs wait legalization (event semaphores), library-load
    # insertion for sparse_gather, and extended-inst ISA lowering.
    nc.compile()
    return nc


def get_nc():
    if "nc" not in _CACHE:
        _CACHE["nc"] = _build_nc()
    return _CACHE["nc"]


def _consts():
    ident = np.eye(128, dtype=np.float32)
    shift = np.zeros((128, 128), dtype=np.float32)
    for p in range(127):
        shift[p, p + 1] = 1.0  # out[j] = in[j-1]
    iota = (np.arange(128)[:, None] * Q + np.arange(Q)[None, :]).astype(np.float32)
    # iota_j[q, f] = f*16 + q (output position in the wrapped layout)
    iotaj = (np.arange(Q)[:, None] + np.arange(128)[None, :] * Q).astype(np.float32)
    return ident, shift, iota, iotaj


def _make_in_maps(x, lengths):
    x = np.ascontiguousarray(np.asarray(x), dtype=np.float32)
    lengths = np.asarray(lengths).astype(np.int32)
    ident, shift, iota, iotaj = _consts()
    in_maps = []
    for i in range(NCORES):
        # [g, b, p, s, a] layout with t = p*16 + g*4 + s: every 4-column tile
        # load is one contiguous 2MB block (16KB runs per partition).
        shard = x[:, i * BL : (i + 1) * BL, :]  # (T, BL, A)
        xr = shard.reshape(P, 2, 8, BL, A).transpose(1, 3, 0, 2, 4)
        in_maps.append(
            {
                "x": np.ascontiguousarray(xr),
                "lengths": np.ascontiguousarray(
                    lengths[i * BL : (i + 1) * BL].reshape(1, BL)
                ),
                "c_ident": ident,
                "c_shift": shift,
                "c_iota": iota,
                "c_iotaj": iotaj,
            }
        )
    return in_maps


def _gather(results):
    tokens = np.concatenate([np.asarray(r["tokens"]) for r in results], axis=0)
    lens = np.concatenate(
        [np.asarray(r["lens_out"]).reshape(BL) for r in results], axis=0
    )
    return tokens.astype(np.int32), lens.astype(np.int32)


def kernel(x, lengths):
    nc = get_nc()
    in_maps = _make_in_maps(x, lengths)
    res = run_bass_kernel_spmd(nc, in_maps, core_ids=list(range(NCORES)))
    return _gather(res.results)


def kernel_profiled(x, lengths, **kw):
    """Like kernel(), but with NTFF profiling; returns (outputs, exec_time_ns)."""
    nc = get_nc()
    in_maps = _make_in_maps(x, lengths)
    res = run_bass_kernel_spmd(
        nc, in_maps, core_ids=list(range(NCORES)), trace=True, **kw
    )
    return _gather(res.results), res.exec_time_ns


# revision 36
# speedup vs baseline: 1.0525x; 1.0525x over previous
"""CTC greedy decoder on 8 TRN2 NeuronCores (Bass/Tile).

Problem: x (2000, 32, 1024) f32 logits, lengths (32,) i32.
Output: tokens (32, 2000) i32 (left-compacted, -1 padded), out_lengths (32,) i32.

Sharding: data-parallel over batch. Each of the 8 cores gets 4 batch rows
(x[:, 4i:4i+4, :], lengths[4i:4i+4]) and decodes them independently.

Per-core algorithm:
  Phase 1 (heavy): argmax over the 1024-wide alphabet for all 2000*4
    positions. The max pass runs on GPSIMD (tensor_reduce), the index
    pass on DVE (max_index with a stride-0 broadcast of the max), and the
    512KB x-tile loads alternate between the two HWDGE rings (SP/ACT).
    Timesteps are tiled as t = p*16 + q (p: 125 partitions, q: 16 tile
    columns per batch row) so "previous timestep" is a free-dim shift.
  Phase 2 (light, per batch row): CTC keep-mask (valid & nonblank &
    non-repeat), mask dropped positions to -1, transpose to the 16-minor
    wrapped layout. The gpsimd.sparse_gather compactions (+ tail -1
    rebuild, since HW leaves garbage past num_found) run after all
    reduces so the GPSIMD library is switched exactly once.
"""

import numpy as np
import sys

sys.path.insert(0, "/opt/trn_rl_repo")

import concourse.bass as bass
import concourse.mybir as mybir
from concourse import bacc
from concourse import dve_ops
from concourse.dve_spec import Spec, Src0, Idx, MaxNeg, select, scan, AluOp
from concourse.dve_spec import lower as dve_lower
from concourse.dve_uop import DveOpSpec
from concourse.tile import TileContext
from concourse.bass_utils import run_bass_kernel_spmd


def _argmax_rev_ref(in0, in1, s0, s1, imm2):
    """CoreSim reference: in0 arrives through the caller's (reversed) AP."""
    y = in0.astype(np.float32).reshape(in0.shape[0], -1)
    r = np.maximum.accumulate(y, axis=1)
    body = np.where(y >= r, np.arange(y.shape[1], dtype=np.float32), -3.4028235e38)
    return body.astype(np.float32), body.max(axis=1, keepdims=True).astype(np.float32)


def _register_argmax_op():
    """One-pass argmax DVE op: stream position of the LAST running-max event.

    Called on a reversed view of the row, the last event is the FIRST
    occurrence of the row maximum in true order, so
    true_argmax = (N-1) - accum_out. Exact for ties, one 1x pass.
    """
    name = "ARGMAX_REV_ANT"
    if any(op.name == name for op in dve_ops.OPS):
        return next(op for op in dve_ops.OPS if op.name == name)
    spec = Spec(
        body=select(Src0 >= scan(AluOp.MAX, Src0), Idx, MaxNeg),
        accum=AluOp.MAX,
        reference=_argmax_rev_ref,
    )
    shas = {}
    for ver in ("v3", "v4"):
        try:
            sp = DveOpSpec(name=name, opcode=0, uops=dve_lower(spec, ver=ver))
            shas[ver] = sp.sha(ver)
        except Exception:
            pass
    op = dve_ops.DveOp(name, spec, subdim=False, uops_sha=shas)
    row = dve_ops._CUSTOM_DVE_ROW_BASE + len(dve_ops.OPS)
    assert row < 0x20
    dve_ops.OPS.append(op)
    dve_ops._SUB_OPCODE_FOR_NAME[name] = row
    dve_ops.CUSTOM_DVE_SPECS[name] = spec
    return op


ARGMAX_REV = _register_argmax_op()

# Problem constants (hardcoded; kernel.py must be self-contained).
T = 2000          # sequence length
B = 32            # global batch
A = 1024          # alphabet size
NCORES = 8
BL = B // NCORES  # batch rows per core = 4
Q = 16            # tile columns per batch row (t = p*Q + q)
P = T // Q        # used partitions per tile = 125

F32 = mybir.dt.float32
I32 = mybir.dt.int32
U32 = mybir.dt.uint32

_CACHE = {}


def _build_nc():
    """Build the single-core Bass graph (same graph runs SPMD on all cores)."""
    nc = bacc.Bacc()

    # x arrives host-rearranged as [g, b, p, s, a] with t = p*16 + g*4 + s,
    # so each 4-column tile load is one fully contiguous 2MB block.
    x_d = nc.declare_dram_parameter("x", [4, BL, P, 4, A], F32, isOutput=False)
    len_d = nc.declare_dram_parameter("lengths", [1, BL], I32, isOutput=False)
    ident_d = nc.declare_dram_parameter("c_ident", [128, 128], F32, isOutput=False)
    shift_d = nc.declare_dram_parameter("c_shift", [128, 128], F32, isOutput=False)
    iota_d = nc.declare_dram_parameter("c_iota", [128, Q], F32, isOutput=False)
    iotaj_d = nc.declare_dram_parameter("c_iotaj", [Q, 128], F32, isOutput=False)
    tok_d = nc.declare_dram_parameter("tokens", [BL, T], I32, isOutput=True)
    lout_d = nc.declare_dram_parameter("lens_out", [BL, 1], I32, isOutput=True)

    xv4 = x_d
    # tokens[b, j] with j = p*16 + q: tokv[b, p, q]
    tokv = tok_d.rearrange("b (p s) -> b p s", s=Q)

    with TileContext(nc) as tc:
        with (
            tc.tile_pool(name="consts", bufs=1) as cpool,
            tc.tile_pool(name="xp", bufs=6) as xpool,
            tc.tile_pool(name="fp", bufs=4) as fpool,
            tc.tile_pool(name="p2", bufs=4) as p2pool,
            tc.tile_pool(name="sg", bufs=4) as sgpool,
            tc.tile_pool(name="ps", bufs=2, space="PSUM") as pspool,
        ):
            # ---- constants ----
            ident = cpool.tile([128, 128], F32)
            nc.sync.dma_start(ident[:], ident_d[:])
            shiftm = cpool.tile([128, 128], F32)
            nc.sync.dma_start(shiftm[:], shift_d[:])
            iota_t = cpool.tile([128, Q], F32)
            nc.sync.dma_start(iota_t[:], iota_d[:])
            neg1 = cpool.tile([128, Q], F32)
            nc.vector.memset(neg1[:], -1.0)
            ones_k = cpool.tile([1, 128], F32)
            nc.vector.memset(ones_k[:], 1.0)
            iota_j = cpool.tile([Q, 128], F32)
            nc.sync.dma_start(iota_j[:], iotaj_d[:])
            neg1w = cpool.tile([Q, 128], F32)
            nc.vector.memset(neg1w[:], -1.0)

            # lengths -> f32, broadcast to all 128 partitions via K=1 matmul
            len_raw = cpool.tile([1, BL], I32)
            nc.sync.dma_start(len_raw[:], len_d[:])
            len_f = cpool.tile([1, BL], F32)
            nc.vector.tensor_copy(len_f[:], len_raw[:])
            len_ps = pspool.tile([128, BL], F32, tag="ps_a")
            nc.tensor.matmul(len_ps[:], ones_k[:], len_f[:])
            len_sb = cpool.tile([128, BL], F32)
            nc.vector.tensor_copy(len_sb[:], len_ps[:])

            sg_ins = []
            for b in range(BL):
                # ---- phase 1: argmax ----
                F = fpool.tile([128, Q], F32)
                nc.vector.memset(F[96:128, :], 0.0)  # covers the 3 unused rows
                for g in range(4):
                    # one SWDGE load brings 4 q-columns as one contiguous 2MB
                    # block, fanning out across all 16 SDMA engines
                    xt4 = xpool.tile([128, 4, A], F32)
                    nc.gpsimd.dma_start(xt4[0:P, :, :], xv4[g, b])
                    for s in range(4):
                        q = g * 4 + s
                        # one-pass argmax: reversed stream, accum = last
                        # running-max event = first max occurrence (reversed)
                        xr = xt4[0:P, s, ::-1]
                        nc.vector._custom_dve(
                            ARGMAX_REV, out=xr, in0=xr,
                            accum_out=F[0:P, q : q + 1],
                        )
                # F holds (A-1) - argmax; flip to true index
                nc.vector.tensor_scalar(
                    F[:], F[:], -1.0, float(A - 1),
                    mybir.AluOpType.mult, mybir.AluOpType.add,
                )

                # ---- phase 2a: CTC keep-mask (DVE/PE only) ----
                # prev of column 0 = F[p-1, 15]: shift matmul
                prev0 = pspool.tile([128, 1], F32, tag="ps_b")
                nc.tensor.matmul(prev0[:], shiftm[:], F[:, Q - 1 : Q])

                neq = p2pool.tile([128, Q], F32)
                nc.vector.tensor_tensor(
                    neq[:, 1:Q], F[:, 1:Q], F[:, 0 : Q - 1], mybir.AluOpType.not_equal
                )
                nc.vector.tensor_tensor(
                    neq[:, 0:1], F[:, 0:1], prev0[:, 0:1], mybir.AluOpType.not_equal
                )
                nc.vector.memset(neq[0:1, 0:1], 1.0)  # t=0: prev is None

                nz = p2pool.tile([128, Q], F32)
                nc.vector.tensor_scalar(
                    nz[:], F[:], 0.0, None, mybir.AluOpType.not_equal
                )
                valid = p2pool.tile([128, Q], F32)
                nc.vector.tensor_tensor(
                    valid[:], iota_t[:],
                    len_sb[:, b : b + 1].broadcast_to([128, Q]),
                    mybir.AluOpType.is_lt,
                )
                keep = p2pool.tile([128, Q], F32)
                nc.vector.tensor_tensor(keep[:], neq[:], nz[:], mybir.AluOpType.mult)
                nc.vector.tensor_tensor(
                    keep[:], keep[:], valid[:], mybir.AluOpType.mult
                )

                keepi = p2pool.tile([128, Q], mybir.dt.uint8)
                nc.vector.tensor_copy(keepi[:], keep[:])
                masked = p2pool.tile([128, Q], F32)
                nc.vector.select(masked[:], keepi[:], F[:], neg1[:])

                # wrap to [16, 128] (t = f*16 + q order)
                tr_ps = pspool.tile([Q, 128], F32, tag="ps_a")
                nc.tensor.transpose(tr_ps[:], masked[:], ident[:])
                sg_in = sgpool.tile([Q, 128], F32, tag=f"sgin{b}")
                nc.vector.tensor_copy(sg_in[:], tr_ps[:])
                sg_ins.append(sg_in)

            # ---- phase 2b: compaction at the end, so sparse_gather never
            # blocks SWDGE load generation on the gpsimd stream ----
            for b in range(BL):
                sg_in = sg_ins[b]
                sg_out = sgpool.tile([Q, 128], F32)
                nf = sgpool.tile([1, 1], U32)
                nc.gpsimd.sparse_gather(sg_out[:], sg_in[:], num_found=nf[:])

                # The HW ucode writes deterministic garbage past num_found
                # (sim pads -1): rebuild the tail as -1 via j < num_found.
                nf_f = sgpool.tile([1, 1], F32)
                nc.vector.tensor_copy(nf_f[:], nf[:])
                nfbc_ps = pspool.tile([Q, 1], F32, tag="ps_b")
                nc.tensor.matmul(nfbc_ps[:], ones_k[0:1, 0:Q], nf_f[:])
                padm = sgpool.tile([Q, 128], mybir.dt.uint8)
                nc.vector.tensor_tensor(
                    padm[:], iota_j[:],
                    nfbc_ps[:, 0:1].broadcast_to([Q, 128]),
                    mybir.AluOpType.is_lt,
                )
                padded = sgpool.tile([Q, 128], F32)
                nc.vector.tensor_copy(padded[:], neg1w[:])
                nc.vector.copy_predicated(padded[:], padm[:], sg_out[:])

                # unwrap back to [128, 16] (j = p*16 + q) and store
                tok_ps = pspool.tile([128, Q], F32, tag="ps_c")
                nc.tensor.transpose(tok_ps[:], padded[:], ident[0:Q, 0:Q])
                toki = p2pool.tile([128, Q], I32)
                nc.vector.tensor_copy(toki[:], tok_ps[:])
                nc.sync.dma_start(tokv[b], toki[0:P, :])

                nf_i = sgpool.tile([1, 1], I32)
                nc.vector.tensor_copy(nf_i[:], nf[:])
                nc.sync.dma_start(lout_d[b : b + 1, :], nf_i[:])

    # Bacc.compile runs wait legalization (event semaphores), library-load
    # insertion for sparse_gather, and extended-inst ISA lowering.
    nc.compile()
    return nc


def get_nc():
    if "nc" not in _CACHE:
        _CACHE["nc"] = _build_nc()
    return _CACHE["nc"]


def _consts():
    ident = np.eye(128, dtype=np.float32)
    shift = np.zeros((128, 128), dtype=np.float32)
    for p in range(127):
        shift[p, p + 1] = 1.0  # out[j] = in[j-1]
    iota = (np.arange(128)[:, None] * Q + np.arange(Q)[None, :]).astype(np.float32)
    # iota_j[q, f] = f*16 + q (output position in the wrapped layout)
    iotaj = (np.arange(Q)[:, None] + np.arange(128)[None, :] * Q).astype(np.float32)
    return ident, shift, iota, iotaj


def _make_in_maps(x, lengths):
    x = np.ascontiguousarray(np.asarray(x), dtype=np.float32)
    lengths = np.asarray(lengths).astype(np.int32)
    ident, shift, iota, iotaj = _consts()
    in_maps = []
    for i in range(NCORES):
        # [g, b, p, s, a] layout with t = p*16 + g*4 + s: every 4-column tile
        # load is one contiguous 2MB block (16KB runs per partition).
        shard = x[:, i * BL : (i + 1) * BL, :]  # (T, BL, A)
        xr = shard.reshape(P, 4, 4, BL, A).transpose(1, 3, 0, 2, 4)
        in_maps.append(
            {
                "x": np.ascontiguousarray(xr),
                "lengths": np.ascontiguousarray(
                    lengths[i * BL : (i + 1) * BL].reshape(1, BL)
                ),
                "c_ident": ident,
                "c_shift": shift,
                "c_iota": iota,
                "c_iotaj": iotaj,
            }
        )
    return in_maps


def _gather(results):
    tokens = np.concatenate([np.asarray(r["tokens"]) for r in results], axis=0)
    lens = np.concatenate(
        [np.asarray(r["lens_out"]).reshape(BL) for r in results], axis=0
    )
    return tokens.astype(np.int32), lens.astype(np.int32)


def kernel(x, lengths):
    nc = get_nc()
    in_maps = _make_in_maps(x, lengths)
    res = run_bass_kernel_spmd(nc, in_maps, core_ids=list(range(NCORES)))
    return _gather(res.results)


def kernel_profiled(x, lengths, **kw):
    """Like kernel(), but with NTFF profiling; returns (outputs, exec_time_ns)."""
    nc = get_nc()
    in_maps = _make_in_maps(x, lengths)
    res = run_bass_kernel_spmd(
        nc, in_maps, core_ids=list(range(NCORES)), trace=True, **kw
    )
    return _gather(res.results), res.exec_time_ns


# revision 37
# speedup vs baseline: 1.2358x; 1.1742x over previous
"""CTC greedy decoder on 8 TRN2 NeuronCores (Bass/Tile).

Problem: x (2000, 32, 1024) f32 logits, lengths (32,) i32.
Output: tokens (32, 2000) i32 (left-compacted, -1 padded), out_lengths (32,) i32.

Sharding: data-parallel over batch. Each of the 8 cores gets 4 batch rows
(x[:, 4i:4i+4, :], lengths[4i:4i+4]) and decodes them independently.

Per-core algorithm:
  Phase 1 (heavy): argmax over the 1024-wide alphabet for all 2000*4
    positions. The max pass runs on GPSIMD (tensor_reduce), the index
    pass on DVE (max_index with a stride-0 broadcast of the max), and the
    512KB x-tile loads alternate between the two HWDGE rings (SP/ACT).
    Timesteps are tiled as t = p*16 + q (p: 125 partitions, q: 16 tile
    columns per batch row) so "previous timestep" is a free-dim shift.
  Phase 2 (light, per batch row): CTC keep-mask (valid & nonblank &
    non-repeat), mask dropped positions to -1, transpose to the 16-minor
    wrapped layout. The gpsimd.sparse_gather compactions (+ tail -1
    rebuild, since HW leaves garbage past num_found) run after all
    reduces so the GPSIMD library is switched exactly once.
"""

import numpy as np
import sys

sys.path.insert(0, "/opt/trn_rl_repo")

import concourse.bass as bass
import concourse.mybir as mybir
from concourse import bacc
from concourse import dve_ops
from concourse.dve_spec import Spec, Src0, Idx, MaxNeg, select, scan, AluOp
from concourse.dve_spec import lower as dve_lower
from concourse.dve_uop import DveOpSpec
from concourse.tile import TileContext
from concourse.bass_utils import run_bass_kernel_spmd


def _argmax_rev_ref(in0, in1, s0, s1, imm2):
    """CoreSim reference: in0 arrives through the caller's (reversed) AP."""
    y = in0.astype(np.float32).reshape(in0.shape[0], -1)
    r = np.maximum.accumulate(y, axis=1)
    body = np.where(y >= r, np.arange(y.shape[1], dtype=np.float32), -3.4028235e38)
    return body.astype(np.float32), body.max(axis=1, keepdims=True).astype(np.float32)


def _register_argmax_op():
    """One-pass argmax DVE op: stream position of the LAST running-max event.

    Called on a reversed view of the row, the last event is the FIRST
    occurrence of the row maximum in true order, so
    true_argmax = (N-1) - accum_out. Exact for ties, one 1x pass.
    """
    name = "ARGMAX_REV_ANT"
    if any(op.name == name for op in dve_ops.OPS):
        return next(op for op in dve_ops.OPS if op.name == name)
    spec = Spec(
        body=select(Src0 >= scan(AluOp.MAX, Src0), Idx, MaxNeg),
        accum=AluOp.MAX,
        reference=_argmax_rev_ref,
    )
    shas = {}
    for ver in ("v3", "v4"):
        try:
            sp = DveOpSpec(name=name, opcode=0, uops=dve_lower(spec, ver=ver))
            shas[ver] = sp.sha(ver)
        except Exception:
            pass
    op = dve_ops.DveOp(name, spec, subdim=False, uops_sha=shas)
    row = dve_ops._CUSTOM_DVE_ROW_BASE + len(dve_ops.OPS)
    assert row < 0x20
    dve_ops.OPS.append(op)
    dve_ops._SUB_OPCODE_FOR_NAME[name] = row
    dve_ops.CUSTOM_DVE_SPECS[name] = spec
    return op


ARGMAX_REV = _register_argmax_op()

# Problem constants (hardcoded; kernel.py must be self-contained).
T = 2000          # sequence length
B = 32            # global batch
A = 1024          # alphabet size
NCORES = 8
BL = B // NCORES  # batch rows per core = 4
Q = 16            # tile columns per batch row (t = p*Q + q)
P = T // Q        # used partitions per tile = 125

F32 = mybir.dt.float32
I32 = mybir.dt.int32
U32 = mybir.dt.uint32

_CACHE = {}


def _build_nc():
    """Build the single-core Bass graph (same graph runs SPMD on all cores)."""
    nc = bacc.Bacc()

    # x arrives host-rearranged as [g, b, p, s, a] with t = p*16 + g*4 + s,
    # so each 4-column tile load is one fully contiguous 2MB block.
    x_d = nc.declare_dram_parameter("x", [4, BL, P, 4, A], F32, isOutput=False)
    len_d = nc.declare_dram_parameter("lengths", [1, BL], I32, isOutput=False)
    ident_d = nc.declare_dram_parameter("c_ident", [128, 128], F32, isOutput=False)
    shift_d = nc.declare_dram_parameter("c_shift", [128, 128], F32, isOutput=False)
    iota_d = nc.declare_dram_parameter("c_iota", [128, Q], F32, isOutput=False)
    iotaj_d = nc.declare_dram_parameter("c_iotaj", [Q, 128], F32, isOutput=False)
    tok_d = nc.declare_dram_parameter("tokens", [BL, T], I32, isOutput=True)
    lout_d = nc.declare_dram_parameter("lens_out", [BL, 1], I32, isOutput=True)

    xv4 = x_d
    # tokens[b, j] with j = p*16 + q: tokv[b, p, q]
    tokv = tok_d.rearrange("b (p s) -> b p s", s=Q)

    with TileContext(nc) as tc:
        with (
            tc.tile_pool(name="consts", bufs=1) as cpool,
            tc.tile_pool(name="xp", bufs=8) as xpool,
            tc.tile_pool(name="fp", bufs=4) as fpool,
            tc.tile_pool(name="p2", bufs=4) as p2pool,
            tc.tile_pool(name="sg", bufs=4) as sgpool,
            tc.tile_pool(name="ps", bufs=2, space="PSUM") as pspool,
        ):
            # ---- issue every x-tile load up front: the gpsimd stream is
            # [16 load-gens][4 sparse_gathers], so DMA streams continuously,
            # throttled only by the xpool slot count ----
            xts = []
            for b in range(BL):
                for g in range(4):
                    xt4 = xpool.tile([128, 4, A], F32)
                    nc.gpsimd.dma_start(xt4[0:P, :, :], xv4[g, b])
                    xts.append(xt4)

            # ---- constants ----
            ident = cpool.tile([128, 128], F32)
            nc.sync.dma_start(ident[:], ident_d[:])
            shiftm = cpool.tile([128, 128], F32)
            nc.sync.dma_start(shiftm[:], shift_d[:])
            iota_t = cpool.tile([128, Q], F32)
            nc.sync.dma_start(iota_t[:], iota_d[:])
            neg1 = cpool.tile([128, Q], F32)
            nc.vector.memset(neg1[:], -1.0)
            ones_k = cpool.tile([1, 128], F32)
            nc.vector.memset(ones_k[:], 1.0)
            iota_j = cpool.tile([Q, 128], F32)
            nc.sync.dma_start(iota_j[:], iotaj_d[:])
            neg1w = cpool.tile([Q, 128], F32)
            nc.vector.memset(neg1w[:], -1.0)

            # lengths -> f32, broadcast to all 128 partitions via K=1 matmul
            len_raw = cpool.tile([1, BL], I32)
            nc.sync.dma_start(len_raw[:], len_d[:])
            len_f = cpool.tile([1, BL], F32)
            nc.vector.tensor_copy(len_f[:], len_raw[:])
            len_ps = pspool.tile([128, BL], F32, tag="ps_a")
            nc.tensor.matmul(len_ps[:], ones_k[:], len_f[:])
            len_sb = cpool.tile([128, BL], F32)
            nc.vector.tensor_copy(len_sb[:], len_ps[:])

            sg_ins = []
            for b in range(BL):
                # ---- phase 1: argmax ----
                F = fpool.tile([128, Q], F32)
                nc.vector.memset(F[96:128, :], 0.0)  # covers the 3 unused rows
                for g in range(4):
                    xt4 = xts[b * 4 + g]
                    for s in range(4):
                        q = g * 4 + s
                        # one-pass argmax: reversed stream, accum = last
                        # running-max event = first max occurrence (reversed)
                        xr = xt4[0:P, s, ::-1]
                        nc.vector._custom_dve(
                            ARGMAX_REV, out=xr, in0=xr,
                            accum_out=F[0:P, q : q + 1],
                        )
                # F holds (A-1) - argmax; flip to true index
                nc.vector.tensor_scalar(
                    F[:], F[:], -1.0, float(A - 1),
                    mybir.AluOpType.mult, mybir.AluOpType.add,
                )

                # ---- phase 2a: CTC keep-mask (DVE/PE only) ----
                # prev of column 0 = F[p-1, 15]: shift matmul
                prev0 = pspool.tile([128, 1], F32, tag="ps_b")
                nc.tensor.matmul(prev0[:], shiftm[:], F[:, Q - 1 : Q])

                neq = p2pool.tile([128, Q], F32)
                nc.vector.tensor_tensor(
                    neq[:, 1:Q], F[:, 1:Q], F[:, 0 : Q - 1], mybir.AluOpType.not_equal
                )
                nc.vector.tensor_tensor(
                    neq[:, 0:1], F[:, 0:1], prev0[:, 0:1], mybir.AluOpType.not_equal
                )
                nc.vector.memset(neq[0:1, 0:1], 1.0)  # t=0: prev is None

                nz = p2pool.tile([128, Q], F32)
                nc.vector.tensor_scalar(
                    nz[:], F[:], 0.0, None, mybir.AluOpType.not_equal
                )
                valid = p2pool.tile([128, Q], F32)
                nc.vector.tensor_tensor(
                    valid[:], iota_t[:],
                    len_sb[:, b : b + 1].broadcast_to([128, Q]),
                    mybir.AluOpType.is_lt,
                )
                keep = p2pool.tile([128, Q], F32)
                nc.vector.tensor_tensor(keep[:], neq[:], nz[:], mybir.AluOpType.mult)
                nc.vector.tensor_tensor(
                    keep[:], keep[:], valid[:], mybir.AluOpType.mult
                )

                keepi = p2pool.tile([128, Q], mybir.dt.uint8)
                nc.vector.tensor_copy(keepi[:], keep[:])
                masked = p2pool.tile([128, Q], F32)
                nc.vector.select(masked[:], keepi[:], F[:], neg1[:])

                # wrap to [16, 128] (t = f*16 + q order)
                tr_ps = pspool.tile([Q, 128], F32, tag="ps_a")
                nc.tensor.transpose(tr_ps[:], masked[:], ident[:])
                sg_in = sgpool.tile([Q, 128], F32, tag=f"sgin{b}")
                nc.vector.tensor_copy(sg_in[:], tr_ps[:])
                sg_ins.append(sg_in)

            # ---- phase 2b: compaction at the end, so sparse_gather never
            # blocks SWDGE load generation on the gpsimd stream ----
            for b in range(BL):
                sg_in = sg_ins[b]
                sg_out = sgpool.tile([Q, 128], F32)
                nf = sgpool.tile([1, 1], U32)
                nc.gpsimd.sparse_gather(sg_out[:], sg_in[:], num_found=nf[:])

                # The HW ucode writes deterministic garbage past num_found
                # (sim pads -1): rebuild the tail as -1 via j < num_found.
                nf_f = sgpool.tile([1, 1], F32)
                nc.vector.tensor_copy(nf_f[:], nf[:])
                nfbc_ps = pspool.tile([Q, 1], F32, tag="ps_b")
                nc.tensor.matmul(nfbc_ps[:], ones_k[0:1, 0:Q], nf_f[:])
                padm = sgpool.tile([Q, 128], mybir.dt.uint8)
                nc.vector.tensor_tensor(
                    padm[:], iota_j[:],
                    nfbc_ps[:, 0:1].broadcast_to([Q, 128]),
                    mybir.AluOpType.is_lt,
                )
                padded = sgpool.tile([Q, 128], F32)
                nc.vector.tensor_copy(padded[:], neg1w[:])
                nc.vector.copy_predicated(padded[:], padm[:], sg_out[:])

                # unwrap back to [128, 16] (j = p*16 + q) and store
                tok_ps = pspool.tile([128, Q], F32, tag="ps_c")
                nc.tensor.transpose(tok_ps[:], padded[:], ident[0:Q, 0:Q])
                toki = p2pool.tile([128, Q], I32)
                nc.vector.tensor_copy(toki[:], tok_ps[:])
                nc.sync.dma_start(tokv[b], toki[0:P, :])

                nf_i = sgpool.tile([1, 1], I32)
                nc.vector.tensor_copy(nf_i[:], nf[:])
                nc.sync.dma_start(lout_d[b : b + 1, :], nf_i[:])

    # Bacc.compile runs wait legalization (event semaphores), library-load
    # insertion for sparse_gather, and extended-inst ISA lowering.
    nc.compile()
    return nc


def get_nc():
    if "nc" not in _CACHE:
        _CACHE["nc"] = _build_nc()
    return _CACHE["nc"]


def _consts():
    ident = np.eye(128, dtype=np.float32)
    shift = np.zeros((128, 128), dtype=np.float32)
    for p in range(127):
        shift[p, p + 1] = 1.0  # out[j] = in[j-1]
    iota = (np.arange(128)[:, None] * Q + np.arange(Q)[None, :]).astype(np.float32)
    # iota_j[q, f] = f*16 + q (output position in the wrapped layout)
    iotaj = (np.arange(Q)[:, None] + np.arange(128)[None, :] * Q).astype(np.float32)
    return ident, shift, iota, iotaj


def _make_in_maps(x, lengths):
    x = np.ascontiguousarray(np.asarray(x), dtype=np.float32)
    lengths = np.asarray(lengths).astype(np.int32)
    ident, shift, iota, iotaj = _consts()
    in_maps = []
    for i in range(NCORES):
        # [g, b, p, s, a] layout with t = p*16 + g*4 + s: every 4-column tile
        # load is one contiguous 2MB block (16KB runs per partition).
        shard = x[:, i * BL : (i + 1) * BL, :]  # (T, BL, A)
        xr = shard.reshape(P, 4, 4, BL, A).transpose(1, 3, 0, 2, 4)
        in_maps.append(
            {
                "x": np.ascontiguousarray(xr),
                "lengths": np.ascontiguousarray(
                    lengths[i * BL : (i + 1) * BL].reshape(1, BL)
                ),
                "c_ident": ident,
                "c_shift": shift,
                "c_iota": iota,
                "c_iotaj": iotaj,
            }
        )
    return in_maps


def _gather(results):
    tokens = np.concatenate([np.asarray(r["tokens"]) for r in results], axis=0)
    lens = np.concatenate(
        [np.asarray(r["lens_out"]).reshape(BL) for r in results], axis=0
    )
    return tokens.astype(np.int32), lens.astype(np.int32)


def kernel(x, lengths):
    nc = get_nc()
    in_maps = _make_in_maps(x, lengths)
    res = run_bass_kernel_spmd(nc, in_maps, core_ids=list(range(NCORES)))
    return _gather(res.results)


def kernel_profiled(x, lengths, **kw):
    """Like kernel(), but with NTFF profiling; returns (outputs, exec_time_ns)."""
    nc = get_nc()
    in_maps = _make_in_maps(x, lengths)
    res = run_bass_kernel_spmd(
        nc, in_maps, core_ids=list(range(NCORES)), trace=True, **kw
    )
    return _gather(res.results), res.exec_time_ns
